# revision 33
# baseline (speedup 1.0000x reference)
"""TRN2 Bass kernel for nn_ConceptEmbeddingConceptPred.

Computes y = concat([einsum('bjd,ijd->bi', x, W_emb) + b_loo,
                     einsum('bjd,hjd->bh', x, W_full) + b_full], axis=1)
where W_emb is the leave-one-out scatter-embedding of W_loo (zero diagonal).

Flattened, this is a (4096 x 16384) @ (16384 x 136) GEMM.

Distribution: contraction(k)-parallel over the 8 cores — core c owns
concepts j in [16c, 16c+16) (k-slice of 2048). Each core computes a full
(136, 4096) partial product; partials are summed on the host (cheap),
bias added, transposed, concatenated.

Final dataflow (fp8e3 x stream, fp16 weights, fp32 PSUM accumulate):
  - x is cast to float8_e3m4 on the host (1.3% rel rms quantization,
    halves DMA to 8.4 MB/core); the PE runs the mixed-dtype matmul
    (fp16 stationary x fp8e3 moving) at the full 1 col/cycle rate.
    Measured rel err 1.37e-2 (deterministic), within the 2e-2 gate.
  - ALL input DMAs and the PE warmup are raw bass emitted BEFORE the
    TileContext, so they execute during the ~7 us framework preamble
    instead of after it. Completion is tracked by two manual counting
    semaphores (one per HWDGE queue, +16 per DMA, FIFO per queue).
  - weights (wl + wf packed as ONE tensor -> one DMA -> one early
    completion ack) are guarded by a wait on the LAST warmup matmul:
    pre-context instructions are never rescheduled, and the NX
    dispatches in order, so that wait provably precedes every
    in-context LDWEIGHTS. 40 warmup matmuls keep the PE busy through
    the HAM activity window so the loo pass starts at the warm 2.4 GHz
    p-state.
  - every loo/full matmul carries an instruction-attached wait for
    exactly the one x DMA rectangle it reads (attached AFTER the tile
    scheduler runs — its internal sim cannot see pre-context DMA
    increments and would report a false deadlock; at runtime the waits
    always satisfy). Matmuls whose PSUM slot reuse already forces a
    much later Tile WAR wait skip the manual one.
  - phase order: loo round 0 (64 back-to-back M=128 matmuls at the
    215 ns/512-col stream floor), loo round 1, then the full-probe
    pass (M=8) in 9 balanced ~455-col chunks on col groups q0/q32/q64
    (3 concurrent streams per k-tile; quadrant 3 is unusable),
    bank-major so each PSUM bank's evacuation overlaps the next bank's
    matmuls. The chunk spanning the round boundary is split so each
    matmul reads one rectangle.
  - all PSUM evacuation on vector (no ACT ops -> no preamble
    ACT_TABLE_LOAD delaying the scalar queue); manual semaphores are
    re-zeroed post-context so repeated NEFF executions stay correct.

History: v5 fp16 baseline 71.7 us -> fp8e3 + phase separation 60.9 ->
pre-context DMA/warmup + guards + balanced full pass: 56.3 us.
"""

import sys

for _p in ("/opt/trn_rl_repo",):
    if _p not in sys.path:
        sys.path.append(_p)

import numpy as np
import ml_dtypes
import concourse.bacc as bacc
import concourse.mybir as mybir
import concourse.tile as tile
from concourse.bass_utils import run_bass_kernel_spmd

dt = mybir.dt

B, C, D, H = 4096, 128, 128, 8
NCORES = 8
JPC = C // NCORES  # 16 concept (= k) tiles per core
KPC = JPC * D  # 2048 contraction elements per core
BCHUNK = 512  # loo batch chunk (fp32 PSUM bank limit)
RCHUNK = 2048  # batch cols per loo round
NR = 2  # loo rounds
NWARM = 40  # raw pre-context warmup matmuls (HAM ramp inside preamble)
NFCH = 9  # full-probe chunks: 3 col groups x 3 streams each, balanced
FEDGE = [round(B * c / NFCH) for c in range(NFCH + 1)]  # chunk edges
NFB = 3  # full-probe PSUM banks (bank w <- chunks 3w..3w+2)

_nc_cache = None


def _build():
    global _nc_cache
    if _nc_cache is not None:
        return _nc_cache

    nc = bacc.Bacc(
        "TRN2", target_bir_lowering=False, debug=False, num_devices=NCORES
    )
    xt_d = nc.dram_tensor(
        "x_t3", (128, JPC, B), dt.float8e3, kind="ExternalInput"
    ).ap()
    # wl and wf packed in one tensor: one DMA -> one completion ack
    w_all_d = nc.dram_tensor(
        "w_all", (D, JPC, C + H), dt.float16, kind="ExternalInput"
    ).ap()
    yl_d = nc.dram_tensor("y_loo_t", (C, B), dt.float16, kind="ExternalOutput").ap()
    yf_d = nc.dram_tensor(
        "y_full_p", (NFB, 128, BCHUNK), dt.float16, kind="ExternalOutput"
    ).ap()

    # ---- raw (pre-TileContext) state ---------------------------------
    x_sb = nc.alloc_sbuf_tensor("x_sb", (128, JPC, B), dt.float8e3).ap()
    w_all = nc.alloc_sbuf_tensor("w_all_sb", (D, JPC, C + H), dt.float16).ap()
    wl = w_all[:, :, :C]
    wf = w_all[:, :, C:]
    warm_w = nc.alloc_sbuf_tensor("warm_w", (128, 128), dt.float16).ap()
    warm_x = nc.alloc_sbuf_tensor("warm_x", (128, 128), dt.float16).ap()
    warm_ps = nc.alloc_psum_tensor("warm_ps", (128, BCHUNK), dt.float32).ap()

    s_sem = nc.alloc_semaphore("xdma_sync")
    c_sem = nc.alloc_semaphore("xdma_scalar")

    # warmup on garbage SBUF contents (results discarded; values never
    # read) — raw back-to-back matmuls pipeline at the PE's native rate.
    # The LAST warmup matmul waits for the wl weight DMA to land: it
    # provably precedes every in-context LDWEIGHTS in the PE FIFO (the
    # matmul-attached waits cannot gate those), and the warmup keeps the
    # PE busy (HAM warm) while the wait drains.
    warm_mms = []
    for _ in range(NWARM):
        warm_mms.append(
            nc.tensor.matmul(
                warm_ps[:, :128], warm_w[:], warm_x[:], start=True, stop=True
            )
        )

    # input DMAs: rectangles of x (strict round-0 then round-1 per FIFO
    # queue, alternating queues for bandwidth balance) + whole weights.
    # Each DMA bumps its queue's semaphore by 16 on completion. wl rides
    # the scalar queue first; the last warmup matmul gates all PE work
    # on its completion.
    cnt = {"s": 0, "c": 0}
    sems = {"s": s_sem, "c": c_sem}
    engs = {"s": nc.sync, "c": nc.scalar}
    rect_r0 = [None] * JPC  # kt -> (sem, threshold) covering round-0 cols
    rect_r1 = [None] * JPC

    def issue(q, k0, k1, a, b, rmap):
        eng = engs[q]
        eng.dma_start(x_sb[:, k0:k1, a:b], xt_d[:, k0:k1, a:b]).then_inc(
            sems[q], 16
        )
        cnt[q] += 1
        for kt in range(k0, k1):
            rmap[kt] = (sems[q], 16 * cnt[q])

    # the single packed weight DMA leads the scalar queue; the last
    # warmup matmul's wait (c_sem >= 16) provably precedes every
    # in-context LDWEIGHTS in the PE FIFO
    nc.scalar.dma_start(w_all[:], w_all_d[:]).then_inc(c_sem, 16)
    cnt["c"] += 1

    # R1(0,2) and kt0-R0 lead the sync queue: their DATA provably lands
    # before the TileContext entry point, so the round-boundary matmuls
    # (whose PSUM-slot-reuse waits the scheduler may or may not emit)
    # can never read them early regardless of schedule
    issue("s", 0, 1, RCHUNK, B, rect_r1)
    issue("s", 0, 1, 0, RCHUNK, rect_r0)
    issue("s", 1, 3, 0, RCHUNK, rect_r0)
    issue("c", 3, 4, 0, RCHUNK, rect_r0)
    issue("s", 4, 6, 0, RCHUNK, rect_r0)
    issue("c", 6, 8, 0, RCHUNK, rect_r0)
    issue("s", 8, 10, 0, RCHUNK, rect_r0)
    issue("c", 10, 12, 0, RCHUNK, rect_r0)
    issue("s", 12, 14, 0, RCHUNK, rect_r0)
    issue("c", 14, 16, 0, RCHUNK, rect_r0)
    issue("c", 1, 4, RCHUNK, B, rect_r1)
    issue("s", 4, 8, RCHUNK, B, rect_r1)
    issue("c", 8, 12, RCHUNK, B, rect_r1)
    issue("s", 12, 16, RCHUNK, B, rect_r1)

    warm_mms[-1]._wait_ge(c_sem, 16)

    def xrect(a, b, kt):
        # the one rectangle containing cols [a, b) of k-tile kt
        assert b <= RCHUNK or a >= RCHUNK
        return rect_r0[kt] if b <= RCHUNK else rect_r1[kt]

    # matmul -> (sem, threshold) waits, attached AFTER the TileContext
    # exits: the tile scheduler's internal sim can't see the pre-context
    # DMA increments (it would report a false deadlock), and the waits
    # always satisfy at runtime since the DMAs are issued unconditionally
    pending_waits = []

    # ---- scheduled (TileContext) program -----------------------------
    with tile.TileContext(nc) as tc:
        with (
            tc.tile_pool(name="ylpool", bufs=2) as ylpool,
            tc.tile_pool(name="yfpool", bufs=2) as yfpool,
            tc.tile_pool(name="psl", bufs=4, space="PSUM") as psl,
            tc.tile_pool(name="psf", bufs=3, space="PSUM") as psf,
        ):
            # loo rounds: 64 back-to-back M=128 matmuls each
            for r in range(NR):
                accs = [
                    psl.tile(
                        [C, BCHUNK], dt.float32, tag="accl", name=f"accl{r}_{c}"
                    )
                    for c in range(4)
                ]
                for kt in range(JPC):
                    for c in range(4):
                        b0 = r * RCHUNK + c * BCHUNK
                        mm = nc.tensor.matmul(
                            accs[c][:],
                            wl[:, kt, :],
                            x_sb[:, kt, b0 : b0 + BCHUNK],
                            start=(kt == 0),
                            stop=(kt == JPC - 1),
                        )
                        pending_waits.append(
                            (mm, xrect(b0, b0 + BCHUNK, kt))
                        )
                yl_sb = ylpool.tile([C, RCHUNK], dt.float16, tag="yl")
                for c in range(4):
                    nc.vector.tensor_copy(
                        yl_sb[:, c * BCHUNK : (c + 1) * BCHUNK], accs[c][:]
                    )
                oeng = nc.sync if r == 0 else nc.scalar
                oeng.dma_start(yl_d[:, r * RCHUNK : (r + 1) * RCHUNK], yl_sb[:])

            # full-probe pass: 9 balanced chunks on col groups q0/q32/q64,
            # bank-major; the round-boundary-spanning chunk is split so
            # every matmul reads exactly one rectangle
            fbanks = [
                psf.tile([128, BCHUNK], dt.float32, tag="accf", name=f"fb{w}")
                for w in range(NFB)
            ]
            for w in range(NFB):
                for kt in range(JPC):
                    for g in range(NFB):
                        c = w * NFB + g
                        a, b = FEDGE[c], FEDGE[c + 1]
                        spans = a < RCHUNK < b
                        pieces = (
                            [(a, RCHUNK), (RCHUNK, b)] if spans else [(a, b)]
                        )
                        for pa, pb in pieces:
                            mm = nc.tensor.matmul(
                                fbanks[w][
                                    32 * g : 32 * g + H, pa - a : pb - a
                                ],
                                wf[:, kt, :],
                                x_sb[:, kt, pa:pb],
                                start=(kt == 0 and pa == a),
                                stop=(kt == JPC - 1 and pb == b),
                            )
                            pending_waits.append((mm, xrect(pa, pb, kt)))
                yf_sb = yfpool.tile([128, BCHUNK], dt.float16, tag="yf")
                # only partition rows [0, 72) hold data (3 groups of 8 at
                # 32-offsets); trimming the copy+DMA shortens the tail
                nc.vector.tensor_copy(yf_sb[:72], fbanks[w][:72])
                oeng = nc.sync if w % 2 == 0 else nc.scalar
                oeng.dma_start(yf_d[w, :72], yf_sb[:72])

    # attach the rect wait unless the scheduler already placed one there
    # (only one wait fits per instruction). Scheduler waits appear on
    # PSUM-slot-reuse matmuls; those matmuls read only the early-landing
    # rects by construction (see the DMA order above), so either way
    # every matmul is safe in every schedule.
    for mm, (sem, thr) in pending_waits:
        hw = mm.ins.has_wait
        if callable(hw):
            hw = hw()
        if not hw:
            mm._wait_ge(sem, thr)

    # re-zero the manual semaphores so back-to-back NEFF executions see
    # the same initial state
    nc.clear_and_free_semaphores([s_sem, c_sem])

    nc.compile()
    _nc_cache = nc
    return nc


def _embed_loo_weights(W_loo):
    # probe i sees concepts j != i; scatter into (C, C, D) with zero row at j=i
    I = np.arange(C)[:, None]
    J = np.arange(C)[None, :]
    src = np.clip(J - (J > I).astype(np.int64), 0, C - 2)  # (C, C)
    W_emb = np.take_along_axis(W_loo, src[:, :, None], axis=1)  # (C, C, D)
    return W_emb * (J != I)[:, :, None].astype(W_loo.dtype)


def _prep_in_maps(x, W_loo, W_full):
    x32 = np.asarray(x, dtype=np.float32)
    # (C, D, B): each core's (JPC, D, B) k-slice is contiguous; then to
    # [d, kt, c] to match the 3D DRAM layout
    xt_all = np.ascontiguousarray(x32.transpose(1, 2, 0)).astype(
        ml_dtypes.float8_e3m4
    )
    W_emb = _embed_loo_weights(np.asarray(W_loo, dtype=np.float32))
    W_full = np.asarray(W_full, dtype=np.float32)
    in_maps = []
    for c in range(NCORES):
        jsl = slice(c * JPC, (c + 1) * JPC)
        xt_c = np.ascontiguousarray(xt_all[jsl].transpose(1, 0, 2))
        # stationary layouts: [d, kt, out] so K (=d) is the partition dim
        wl_c = W_emb[:, jsl, :].transpose(2, 1, 0).astype(np.float16)
        wf_c = W_full[:, jsl, :].transpose(2, 1, 0).astype(np.float16)
        w_all_c = np.ascontiguousarray(np.concatenate([wl_c, wf_c], axis=2))
        in_maps.append({"x_t3": xt_c, "w_all": w_all_c})
    return in_maps


def _assemble(results, b_loo, b_full):
    y_loo_t = np.zeros((C, B), np.float64)
    y_full_t = np.zeros((H, B), np.float64)
    for res in results:
        y_loo_t += res["y_loo_t"]
        yf_p = res["y_full_p"]  # (NFB, 128, BCHUNK) packed col groups
        for c in range(NFCH):
            w, g = divmod(c, NFB)
            a, b = FEDGE[c], FEDGE[c + 1]
            y_full_t[:, a:b] += yf_p[w, 32 * g : 32 * g + H, : b - a]
    y_loo = (y_loo_t.T + np.asarray(b_loo, np.float64)[None, :]).astype(np.float32)
    y_full = (y_full_t.T + np.asarray(b_full, np.float64)[None, :]).astype(np.float32)
    return np.concatenate([y_loo, y_full], axis=1)


def run_spmd(x, W_loo, b_loo, W_full, b_full, trace=False):
    nc = _build()
    in_maps = _prep_in_maps(x, W_loo, W_full)
    res = run_bass_kernel_spmd(
        nc, in_maps, core_ids=list(range(NCORES)), trace=trace
    )
    return _assemble(res.results, b_loo, b_full), res


def kernel(x, W_loo, b_loo, W_full, b_full):
    out, _ = run_spmd(x, W_loo, b_loo, W_full, b_full)
    return out
